# revision 11
# baseline (speedup 1.0000x reference)
"""MoE expert-routing kernel for Trainium2 (8 NeuronCores, expert-parallel).

Problem: out[t] = x[t] @ weight[index[t]] + bias[index[t]]
  x: (32768, 512) f32, index: (32768,) int, weight: (8, 512, 512) f32,
  bias: (8, 512) f32.

Strategy (expert-parallel, host-side dispatch):
  Core e owns expert e. The host gathers the tokens routed to expert e
  into a fixed-capacity, transposed buffer xt_e[512, CAP] (padded with
  zeros), and core e computes y_e = x_e @ W_e + b_e as a single dense
  GEMM. Results are scattered back to token order on the host. Tokens
  beyond CAP fall back to a host matmul, so the kernel stays correct
  for any index distribution.

Device kernel (per core): y = x_e @ W_e + b_e over CAP=4224 tokens
  - Host packs x_e pre-transposed AND slab-contiguous (single
    contiguous run per partition per slab DMA; no strided descriptors).
  - Measurement model (from NTFF traces): exec_time_ns counts from the
    framework's first constant-memset (~5.9us; the runtime-go wait and
    engine barriers before it are EXCLUDED) to the end of the last
    instruction, which includes a runtime-injected ~6.7us postamble
    that zeroes the whole 256-entry semaphore file one EVENT_SEMAPHORE
    per sem across 5 engines. Both ends are fixed framework cost
    (~9.3us counted); the optimizable window is user work.
  - Startup (v3): all head pieces ride the SP ring in consumption
    order ([xs0|w0], w1, w2, w3, then the x slabs); pieces land at
    wire rate (~0.35us/128KB) while the cold-clock first-tile MMs
    consume at ~0.43us/MM, so the GEMM starts on piece A (~9.6us)
    instead of on the whole 640KB head (~10.7us). The scratch memset
    runs on DVE (free at ~6.9us vs GpSimd's ~7.7us) and the warmup is
    sized to END at piece-A-ready - ending early is harmless, ending
    late delays the GEMM 1:1. The first ~5 real MMs run at 1.2GHz
    (the HAM clock-gate warms ~5us after the first warmup MM,
    regardless); that half-rate tax is smaller than waiting.
  - PE p-state: the HAM clock gate runs the PE at 1.2GHz until it has
    been busy ~3.4us, then 2.4GHz. Warmup matmuls on a memset scratch
    tile bridge the DMA-visibility window so the real GEMM runs fully
    ramped at ~216ns per [128x128]@[128x512] fp16 matmul - the PE
    roofline.
  - Token slabs (128/128/256 ramp-in, 512 steady, 384/128/128 tail)
    stream through SBUF on the SP ring; per 128-token tile, 4
    accumulating matmuls into one PSUM bank; DVE adds the bias while
    moving PSUM->SBUF. Outputs ride the ACT ring except the
    second-to-last slab and the final half (idle SP ring by then).
    The last tile is computed as two half-width accumulation groups in
    recycled warmup-PSUM tiles so the first half's bias-add and store
    launch before the last matmul retires, and the final transfer
    overlaps the second half's add.
  - Operands and output are fp16 (values are O(1); PSUM accumulation
    stays fp32): absmax 2.7e-3 on scale-5.5 outputs (4.9e-4 relative).
"""

import os

import numpy as np

N_EXPERTS = 8
D_IN = 512
D_OUT = 512
N_TOKENS = 32768
CAP = 4224  # per-expert token capacity: 33*128; host fallback covers overflow
TOK_SLAB = 512
KC = D_IN // 128  # 4 contraction chunks

# Warmup sizing: starts ~7.4us (DVE memset right after the framework
# preamble), should end at first-data-ready (~9.6us). Ending EARLY is
# harmless (the PE idles <1us, far below the HAM MID re-throttle
# window); ending late delays the GEMM 1:1. Cold-rate costs: 512-col
# MM ~427ns, 128-col MM ~107ns.
WARM512 = int(os.environ.get("KERNEL_WARM512", "5"))
WARM128 = int(os.environ.get("KERNEL_WARM128", "0"))


def _slab_schedule():
    head_sizes = [128, 128, 256]
    tail_sizes = [128, 128]
    sizes = list(head_sizes)
    remaining = CAP - sum(head_sizes) - sum(tail_sizes)
    while remaining > 0:
        sizes.append(min(TOK_SLAB, remaining))
        remaining -= sizes[-1]
    sizes.extend(tail_sizes)
    slabs = []
    t0 = 0
    for ts in sizes:
        slabs.append((t0, ts))
        t0 += ts
    assert t0 == CAP
    return slabs


SLABS = _slab_schedule()
Y_FREE = (CAP // 128) * D_OUT  # packed output free size per partition
HEAD_TOK = SLABS[0][1]  # tokens in slab 0 (rides in the head pack)
HEADA_FREE = KC * HEAD_TOK + D_OUT  # [xs0 | w0]: everything tile0/k0 needs

# mode -> (x dtype, w dtype, y dtype); x and w must match (packed DMAs).
MM_DTYPE = os.environ.get("KERNEL_MM_DTYPE", "float16_o16")
_DT_MAP = {
    "float32": ("float32", "float32", "float32"),
    "float32r": ("float32r", "float32r", "float32"),
    "float32r_o16": ("float32r", "float32r", "float16"),
    "bfloat16": ("bfloat16", "bfloat16", "float32"),
    "float16": ("float16", "float16", "float32"),
    "float16_o16": ("float16", "float16", "float16"),
}

_cache = {}


def _build(mm_dtype_name):
    import concourse.bacc as bacc
    import concourse.mybir as mybir
    import concourse.tile as tile

    x_dt_name, w_dt_name, y_dt_name = _DT_MAP[mm_dtype_name]
    assert x_dt_name == w_dt_name
    dt_x = getattr(mybir.dt, x_dt_name)
    dt_y = getattr(mybir.dt, y_dt_name)
    f32 = mybir.dt.float32

    nc = bacc.Bacc("TRN2", target_bir_lowering=False, debug=False, num_devices=N_EXPERTS)
    # Slab-contiguous packed layouts: one contiguous run per partition
    # per slab DMA. head1 = [xs_slab0 | w0 | w1]; head2 = [w2 | w3];
    # xt's slab-0 region is unused (kept so the host packer stays
    # uniform).
    xt = nc.dram_tensor("xt", (128, KC * CAP), dt_x, kind="ExternalInput").ap()
    head_a = nc.dram_tensor("head_a", (128, HEADA_FREE), dt_x, kind="ExternalInput").ap()
    head_b = nc.dram_tensor("head_b", (128, D_OUT), dt_x, kind="ExternalInput").ap()
    head_c = nc.dram_tensor("head_c", (128, D_OUT), dt_x, kind="ExternalInput").ap()
    head_d = nc.dram_tensor("head_d", (128, D_OUT), dt_x, kind="ExternalInput").ap()
    b = nc.dram_tensor("b", (128, D_OUT), dt_x, kind="ExternalInput").ap()
    y = nc.dram_tensor("y", (128, Y_FREE), dt_y, kind="ExternalOutput").ap()

    with tile.TileContext(nc) as tc:
        with (
            tc.tile_pool(name="wpool", bufs=1) as wpool,
            tc.tile_pool(name="wpoolb", bufs=1) as wpoolb,
            tc.tile_pool(name="wpoolc", bufs=1) as wpoolc,
            tc.tile_pool(name="wpoold", bufs=1) as wpoold,
            tc.tile_pool(name="bias", bufs=1) as bias_pool,
            tc.tile_pool(name="warm", bufs=1) as warm_pool,
            tc.tile_pool(name="xslab", bufs=4) as xpool,
            tc.tile_pool(name="ystage", bufs=8) as ypool,
            tc.tile_pool(name="psum", bufs=6, space="PSUM") as pspool,
            tc.tile_pool(name="wpsum", bufs=2, space="PSUM") as warm_ps_pool,
        ):
            slabs = SLABS

            # Startup DMAs first, all on the SP ring in consumption
            # order ([xs0|w0], then w1, w2, w3): the pieces land one
            # after another at wire rate (~0.35us per 128KB) while the
            # first-tile matmuls consume them at the cold-clock rate
            # (~0.43us per MM) - a gapless pipeline that starts the
            # GEMM ~0.7us before the last piece lands. The ACT ring
            # (whose HWDGE kickoff was measured ~1.6us slower than
            # SP's) carries only the bias, needed much later.
            ha_sb = wpool.tile([128, HEADA_FREE], dt_x, tag="ha", name="ha_sb")
            hb_sb = wpoolb.tile([128, D_OUT], dt_x, tag="hb", name="hb_sb")
            hc_sb = wpoolc.tile([128, D_OUT], dt_x, tag="hc", name="hc_sb")
            hd_sb = wpoold.tile([128, D_OUT], dt_x, tag="hd", name="hd_sb")
            b_rep = bias_pool.tile([128, D_OUT], dt_x, tag="brep")
            nc.sync.dma_start(ha_sb[:], head_a[:])
            nc.sync.dma_start(hb_sb[:], head_b[:])
            nc.sync.dma_start(hc_sb[:], head_c[:])
            nc.sync.dma_start(hd_sb[:], head_d[:])
            nc.scalar.dma_start(b_rep[:], b[:])

            # PE p-state warmup (see module docstring). Scratch memset on
            # DVE - it is free right after the framework preamble, so the
            # warmup chain starts ~0.4us earlier than a GpSimd memset.
            scratch = warm_pool.tile([128, D_OUT], dt_x, tag="scr")
            nc.vector.memset(scratch[:], 0.0)
            wps_a = warm_ps_pool.tile([128, D_OUT], f32, tag="wacc")
            for i in range(WARM512):
                nc.tensor.matmul(
                    wps_a[:], scratch[:, 0:128], scratch[:],
                    start=(i == 0), stop=(i == WARM512 - 1),
                )
            wps_b = warm_ps_pool.tile([128, D_OUT], f32, tag="wacc")
            for i in range(WARM128):
                nc.tensor.matmul(
                    wps_b[:, 0:128], scratch[:, 0:128], scratch[:, 0:128],
                    start=(i == 0), stop=(i == WARM128 - 1),
                )

            xs0_off = KC * HEAD_TOK
            w_aps = [
                ha_sb[:, xs0_off : xs0_off + D_OUT],
                hb_sb[:],
                hc_sb[:],
                hd_sb[:],
            ]

            def load_x(slab_i):
                t0, ts = slabs[slab_i]
                xs = xpool.tile([128, KC * ts], dt_x, tag="xs")
                nc.sync.dma_start(xs[:], xt[:, KC * t0 : KC * (t0 + ts)])
                return xs

            xs_pending = load_x(1)

            n_slabs = len(slabs)
            for i, (t0, ts) in enumerate(slabs):
                nt = ts // 128
                if i == 0:
                    xs = ha_sb[:, 0:xs0_off]
                else:
                    xs = xs_pending[:]
                    if i + 1 < n_slabs:
                        xs_pending = load_x(i + 1)
                ys = ypool.tile([128, nt * D_OUT], dt_y, tag="ys")
                last = i == n_slabs - 1
                o0 = (t0 // 128) * D_OUT
                if last:
                    # Final tile: two half-width accumulation groups (8x
                    # 256-col matmuls, same PE cost) in recycled warmup
                    # PSUM tiles, so the first half's bias-add and store
                    # launch ~0.43us before the last matmul retires and
                    # the final transfer overlaps the second half's add.
                    h = D_OUT // 2
                    ts_off = [k * ts for k in range(KC)]
                    ps_h1 = warm_ps_pool.tile([128, D_OUT], f32, tag="wacc")
                    for k in range(KC):
                        nc.tensor.matmul(
                            ps_h1[:, 0:h],
                            xs[:, ts_off[k] : ts_off[k] + 128],
                            w_aps[k][:, 0:h],
                            start=(k == 0),
                            stop=(k == KC - 1),
                        )
                    nc.vector.tensor_add(
                        ys[:, 0:h], ps_h1[:, 0:h], b_rep[:, 0:h]
                    )
                    nc.scalar.dma_start(y[:, o0 : o0 + h], ys[:, 0:h])
                    ps_h2 = warm_ps_pool.tile([128, D_OUT], f32, tag="wacc")
                    for k in range(KC):
                        nc.tensor.matmul(
                            ps_h2[:, 0:h],
                            xs[:, ts_off[k] : ts_off[k] + 128],
                            w_aps[k][:, h:D_OUT],
                            start=(k == 0),
                            stop=(k == KC - 1),
                        )
                    nc.vector.tensor_add(
                        ys[:, h:D_OUT], ps_h2[:, 0:h], b_rep[:, h:D_OUT]
                    )
                    nc.sync.dma_start(
                        y[:, o0 + h : o0 + D_OUT], ys[:, h:D_OUT]
                    )
                    continue
                for a in range(nt):
                    ps = pspool.tile([128, D_OUT], f32, tag="acc")
                    for k in range(KC):
                        nc.tensor.matmul(
                            ps[:],
                            xs[:, k * ts + a * 128 : k * ts + (a + 1) * 128],
                            w_aps[k],
                            start=(k == 0),
                            stop=(k == KC - 1),
                        )
                    nc.vector.tensor_add(
                        ys[:, a * D_OUT : (a + 1) * D_OUT], ps[:], b_rep[:]
                    )
                if not last:
                    # Alternate outputs across both HWDGE rings: halves
                    # each ring's FIFO depth so the final pieces sit
                    # near the queue head when the kernel drains.
                    # (Ring choice doesn't change aggregate wire BW -
                    # both rings share the 16 SDMA engines - and
                    # interleaved outputs queue BEHIND the input slabs
                    # already in the SP FIFO, so input arrival is
                    # unaffected.)
                    eng = nc.sync if i % 2 == 0 else nc.scalar
                    eng.dma_start(y[:, o0 : o0 + nt * D_OUT], ys[:])
    nc.compile()
    return nc


def _get_nc(mm_dtype_name):
    if mm_dtype_name not in _cache:
        _cache[mm_dtype_name] = _build(mm_dtype_name)
    return _cache[mm_dtype_name]


def kernel(x, index, weight, bias, _trace=False):
    from concourse.bass_utils import run_bass_kernel_spmd

    x = np.ascontiguousarray(np.asarray(x, dtype=np.float32))
    weight = np.ascontiguousarray(np.asarray(weight, dtype=np.float32))
    bias = np.ascontiguousarray(np.asarray(bias, dtype=np.float32))
    idx = np.asarray(index).astype(np.int64, copy=False)

    ids = [np.nonzero(idx == e)[0] for e in range(N_EXPERTS)]

    in_maps = []
    for e in range(N_EXPERTS):
        n_e = min(len(ids[e]), CAP)
        x_e = np.zeros((CAP, D_IN), dtype=np.float32)
        x_e[:n_e] = x[ids[e][:n_e]]
        # Pack slab-major: xt_e[p, KC*t0 + kc*ts + t] = x_e[t0+t, kc*128+p]
        xt_e = np.empty((128, KC * CAP), dtype=np.float32)
        for t0, ts in SLABS:
            blk = x_e[t0 : t0 + ts].reshape(ts, KC, 128)  # [t, kc, p]
            xt_e[:, KC * t0 : KC * (t0 + ts)] = (
                blk.transpose(2, 1, 0).reshape(128, KC * ts)
            )
        w_e = weight[e]
        head_a_e = np.concatenate(
            [xt_e[:, 0 : KC * HEAD_TOK], w_e[0:128, :]], axis=1
        )
        in_maps.append(
            {
                "xt": xt_e,
                "head_a": np.ascontiguousarray(head_a_e),
                "head_b": np.ascontiguousarray(w_e[128:256, :]),
                "head_c": np.ascontiguousarray(w_e[256:384, :]),
                "head_d": np.ascontiguousarray(w_e[384:512, :]),
                "b": np.ascontiguousarray(
                    np.broadcast_to(bias[e], (128, D_OUT))
                ),
            }
        )

    x_dt_name, _, _ = _DT_MAP[MM_DTYPE]
    cast = {"bfloat16": None, "float16": np.float16, "float32": np.float32,
            "float32r": np.float32}
    ct = cast[x_dt_name]
    if ct is None:
        import ml_dtypes

        ct = ml_dtypes.bfloat16
    in_maps = [
        {k: v.astype(ct) for k, v in m.items()}
        for m in in_maps
    ]

    nc = _get_nc(MM_DTYPE)
    res = run_bass_kernel_spmd(
        nc, in_maps, core_ids=list(range(N_EXPERTS)), trace=_trace
    )

    out = np.empty((x.shape[0], D_OUT), dtype=np.float32)
    for e in range(N_EXPERTS):
        n_e = min(len(ids[e]), CAP)
        # Unpack [p, a_global, o] -> token-major [a_global*128+p, o]
        y_pm = res.results[e]["y"].reshape(128, CAP // 128, D_OUT)
        y_e = y_pm.transpose(1, 0, 2).reshape(CAP, D_OUT)
        out[ids[e][:n_e]] = y_e[:n_e].astype(np.float32)
        if len(ids[e]) > CAP:  # capacity overflow: host fallback (correctness net)
            over = ids[e][CAP:]
            out[over] = x[over] @ weight[e] + bias[e]

    if _trace:
        return out, res

    return out


# revision 18
# speedup vs baseline: 1.0517x; 1.0517x over previous
"""MoE expert-routing kernel for Trainium2 (8 NeuronCores, expert-parallel).

Problem: out[t] = x[t] @ weight[index[t]] + bias[index[t]]
  x: (32768, 512) f32, index: (32768,) int, weight: (8, 512, 512) f32,
  bias: (8, 512) f32.

Strategy (expert-parallel, host-side dispatch):
  Core e owns expert e. The host gathers the tokens routed to expert e
  into a fixed-capacity, transposed buffer xt_e[512, CAP] (padded with
  zeros), and core e computes y_e = x_e @ W_e + b_e as a single dense
  GEMM. Results are scattered back to token order on the host. Tokens
  beyond CAP fall back to a host matmul, so the kernel stays correct
  for any index distribution.

Device kernel (per core): y = x_e @ W_e + b_e over CAP=4224 tokens
  - Host packs x_e pre-transposed AND slab-contiguous (single
    contiguous run per partition per slab DMA; no strided descriptors).
  - Measurement model (from NTFF traces): exec_time_ns counts from the
    framework's first constant-memset (~5.9us; the runtime-go wait and
    engine barriers before it are EXCLUDED) to the end of the last
    instruction, which includes a runtime-injected ~6.7us postamble
    that zeroes the whole 256-entry semaphore file one EVENT_SEMAPHORE
    per sem across 5 engines. Both ends are fixed framework cost
    (~9.3us counted); the optimizable window is user work.
  - Startup (v4): two head pieces, both on the SP ring in consumption
    order: [xs0|w0|w1] (384KB, sem ~10.1us) then [w2|w3] (256KB,
    ~11.0us); the GEMM starts on piece A while B streams, and B lands
    just before tile 0's k=2 needs it at the cold-clock rate. Finer
    staging was tried and LOST: sub-256KB DMAs pay enough
    per-transfer overhead to halve the effective wire rate. The
    scratch memset runs on DVE (free at ~6.9us vs GpSimd's ~7.7us)
    and the warmup is sized to END at piece-A-ready - ending early is
    harmless, ending late delays the GEMM 1:1. The first ~5 real MMs
    run at 1.2GHz (the HAM clock-gate warms ~5us after the first
    warmup MM regardless); that half-rate tax is smaller than
    waiting.
  - PE p-state: the HAM clock gate runs the PE at 1.2GHz until it has
    been busy ~3.4us, then 2.4GHz. Warmup matmuls on a memset scratch
    tile bridge the DMA-visibility window so the real GEMM runs fully
    ramped at ~216ns per [128x128]@[128x512] fp16 matmul - the PE
    roofline.
  - Token slabs (128/128/256 ramp-in, 512 steady, 384/128/128 tail)
    stream through SBUF on the SP ring; per 128-token tile, 4
    accumulating matmuls into one PSUM bank; DVE adds the bias while
    moving PSUM->SBUF. Outputs ride the ACT ring except the
    second-to-last slab and the final half (idle SP ring by then).
    The last tile is computed as two half-width accumulation groups in
    recycled warmup-PSUM tiles so the first half's bias-add and store
    launch before the last matmul retires, and the final transfer
    overlaps the second half's add.
  - Operands and output are fp16 (values are O(1); PSUM accumulation
    stays fp32): absmax 2.7e-3 on scale-5.5 outputs (4.9e-4 relative).
"""

import os

import numpy as np

N_EXPERTS = 8
D_IN = 512
D_OUT = 512
N_TOKENS = 32768
CAP = 4224  # per-expert token capacity: 33*128; host fallback covers overflow
TOK_SLAB = 512
KC = D_IN // 128  # 4 contraction chunks

# Warmup sizing: starts ~7.4us (DVE memset right after the framework
# preamble), should end at first-data-ready (~9.6us). Ending EARLY is
# harmless (the PE idles <1us, far below the HAM MID re-throttle
# window); ending late delays the GEMM 1:1. Cold-rate costs: 512-col
# MM ~427ns, 128-col MM ~107ns.
WARM512 = int(os.environ.get("KERNEL_WARM512", "6"))
WARM128 = int(os.environ.get("KERNEL_WARM128", "2"))


def _slab_schedule():
    head_sizes = [128, 128, 256]
    tail_sizes = [128, 128]
    sizes = list(head_sizes)
    remaining = CAP - sum(head_sizes) - sum(tail_sizes)
    while remaining > 0:
        sizes.append(min(TOK_SLAB, remaining))
        remaining -= sizes[-1]
    sizes.extend(tail_sizes)
    slabs = []
    t0 = 0
    for ts in sizes:
        slabs.append((t0, ts))
        t0 += ts
    assert t0 == CAP
    return slabs


SLABS = _slab_schedule()
Y_FREE = (CAP // 128) * D_OUT  # packed output free size per partition
HEAD_TOK = SLABS[0][1]  # tokens in slab 0 (rides in the head pack)
HEADA_FREE = KC * HEAD_TOK + 2 * D_OUT  # [xs0 | w0 | w1]
HEADB_FREE = 2 * D_OUT  # [w2 | w3]

# mode -> (x dtype, w dtype, y dtype); x and w must match (packed DMAs).
MM_DTYPE = os.environ.get("KERNEL_MM_DTYPE", "float16_o16")
_DT_MAP = {
    "float32": ("float32", "float32", "float32"),
    "float32r": ("float32r", "float32r", "float32"),
    "float32r_o16": ("float32r", "float32r", "float16"),
    "bfloat16": ("bfloat16", "bfloat16", "float32"),
    "float16": ("float16", "float16", "float32"),
    "float16_o16": ("float16", "float16", "float16"),
}

_cache = {}


def _build(mm_dtype_name):
    import concourse.bacc as bacc
    import concourse.mybir as mybir
    import concourse.tile as tile

    x_dt_name, w_dt_name, y_dt_name = _DT_MAP[mm_dtype_name]
    assert x_dt_name == w_dt_name
    dt_x = getattr(mybir.dt, x_dt_name)
    dt_y = getattr(mybir.dt, y_dt_name)
    f32 = mybir.dt.float32

    nc = bacc.Bacc("TRN2", target_bir_lowering=False, debug=False, num_devices=N_EXPERTS)
    # Slab-contiguous packed layouts: one contiguous run per partition
    # per slab DMA. head1 = [xs_slab0 | w0 | w1]; head2 = [w2 | w3];
    # xt's slab-0 region is unused (kept so the host packer stays
    # uniform).
    xt = nc.dram_tensor("xt", (128, KC * CAP), dt_x, kind="ExternalInput").ap()
    head_a = nc.dram_tensor("head_a", (128, HEADA_FREE), dt_x, kind="ExternalInput").ap()
    head_b = nc.dram_tensor("head_b", (128, HEADB_FREE), dt_x, kind="ExternalInput").ap()
    b = nc.dram_tensor("b", (128, D_OUT), dt_x, kind="ExternalInput").ap()
    y = nc.dram_tensor("y", (128, Y_FREE), dt_y, kind="ExternalOutput").ap()

    with tile.TileContext(nc) as tc:
        with (
            tc.tile_pool(name="wpool", bufs=1) as wpool,
            tc.tile_pool(name="wpoolb", bufs=1) as wpoolb,
            tc.tile_pool(name="bias", bufs=1) as bias_pool,
            tc.tile_pool(name="warm", bufs=1) as warm_pool,
            tc.tile_pool(name="xslab", bufs=5) as xpool,
            tc.tile_pool(name="ystage", bufs=8) as ypool,
            tc.tile_pool(name="psum", bufs=6, space="PSUM") as pspool,
            tc.tile_pool(name="wpsum", bufs=2, space="PSUM") as warm_ps_pool,
        ):
            slabs = SLABS

            # Startup DMAs first, both on the SP ring (deterministic
            # ~1.4us kickoff; the ACT ring's was measured 1.5-3.6us) in
            # consumption order: [xs0|w0|w1] (384KB, ~10.1us) then
            # [w2|w3] (256KB, ~11.0us). Tile 0's k=0,1 MMs start on
            # piece A at the cold-clock rate (~0.43us/MM), and piece B
            # lands just before k=2 needs it. Finer staging loses:
            # sub-256KB DMAs pay enough per-transfer overhead that the
            # effective wire rate halves. The ACT ring carries only the
            # bias (needed ~1us later than piece B).
            ha_sb = wpool.tile([128, HEADA_FREE], dt_x, tag="ha", name="ha_sb")
            hb_sb = wpoolb.tile([128, HEADB_FREE], dt_x, tag="hb", name="hb_sb")
            b_rep = bias_pool.tile([128, D_OUT], dt_x, tag="brep")
            nc.sync.dma_start(ha_sb[:], head_a[:])
            nc.sync.dma_start(hb_sb[:], head_b[:])
            nc.scalar.dma_start(b_rep[:], b[:])

            # PE p-state warmup (see module docstring). Scratch memset on
            # DVE - it is free right after the framework preamble, so the
            # warmup chain starts ~0.4us earlier than a GpSimd memset.
            scratch = warm_pool.tile([128, D_OUT], dt_x, tag="scr")
            nc.vector.memset(scratch[:], 0.0)
            wps_a = warm_ps_pool.tile([128, D_OUT], f32, tag="wacc")
            for i in range(WARM512):
                nc.tensor.matmul(
                    wps_a[:], scratch[:, 0:128], scratch[:],
                    start=(i == 0), stop=(i == WARM512 - 1),
                )
            wps_b = warm_ps_pool.tile([128, D_OUT], f32, tag="wacc")
            for i in range(WARM128):
                nc.tensor.matmul(
                    wps_b[:, 0:128], scratch[:, 0:128], scratch[:, 0:128],
                    start=(i == 0), stop=(i == WARM128 - 1),
                )

            xs0_off = KC * HEAD_TOK
            w_aps = [
                ha_sb[:, xs0_off : xs0_off + D_OUT],
                ha_sb[:, xs0_off + D_OUT : xs0_off + 2 * D_OUT],
                hb_sb[:, 0:D_OUT],
                hb_sb[:, D_OUT : 2 * D_OUT],
            ]

            def load_x(slab_i):
                t0, ts = slabs[slab_i]
                xs = xpool.tile([128, KC * ts], dt_x, tag="xs")
                nc.sync.dma_start(xs[:], xt[:, KC * t0 : KC * (t0 + ts)])
                return xs

            xs_pending = load_x(1)

            n_slabs = len(slabs)
            for i, (t0, ts) in enumerate(slabs):
                nt = ts // 128
                if i == 0:
                    xs = ha_sb[:, 0:xs0_off]
                else:
                    xs = xs_pending[:]
                    if i + 1 < n_slabs:
                        xs_pending = load_x(i + 1)
                ys = ypool.tile([128, nt * D_OUT], dt_y, tag="ys")
                last = i == n_slabs - 1
                o0 = (t0 // 128) * D_OUT
                if last:
                    # Final tile: two half-width accumulation groups (8x
                    # 256-col matmuls, same PE cost) in recycled warmup
                    # PSUM tiles, so the first half's bias-add and store
                    # launch ~0.43us before the last matmul retires and
                    # the final transfer overlaps the second half's add.
                    h = D_OUT // 2
                    ts_off = [k * ts for k in range(KC)]
                    ps_h1 = warm_ps_pool.tile([128, D_OUT], f32, tag="wacc")
                    for k in range(KC):
                        nc.tensor.matmul(
                            ps_h1[:, 0:h],
                            xs[:, ts_off[k] : ts_off[k] + 128],
                            w_aps[k][:, 0:h],
                            start=(k == 0),
                            stop=(k == KC - 1),
                        )
                    nc.vector.tensor_add(
                        ys[:, 0:h], ps_h1[:, 0:h], b_rep[:, 0:h]
                    )
                    nc.scalar.dma_start(y[:, o0 : o0 + h], ys[:, 0:h])
                    ps_h2 = warm_ps_pool.tile([128, D_OUT], f32, tag="wacc")
                    for k in range(KC):
                        nc.tensor.matmul(
                            ps_h2[:, 0:h],
                            xs[:, ts_off[k] : ts_off[k] + 128],
                            w_aps[k][:, h:D_OUT],
                            start=(k == 0),
                            stop=(k == KC - 1),
                        )
                    nc.vector.tensor_add(
                        ys[:, h:D_OUT], ps_h2[:, 0:h], b_rep[:, h:D_OUT]
                    )
                    nc.sync.dma_start(
                        y[:, o0 + h : o0 + D_OUT], ys[:, h:D_OUT]
                    )
                    continue
                for a in range(nt):
                    ps = pspool.tile([128, D_OUT], f32, tag="acc")
                    for k in range(KC):
                        nc.tensor.matmul(
                            ps[:],
                            xs[:, k * ts + a * 128 : k * ts + (a + 1) * 128],
                            w_aps[k],
                            start=(k == 0),
                            stop=(k == KC - 1),
                        )
                    nc.vector.tensor_add(
                        ys[:, a * D_OUT : (a + 1) * D_OUT], ps[:], b_rep[:]
                    )
                if not last:
                    # Outputs ride the ACT HWDGE ring; the second-to-last
                    # slab uses the idle SP ring. (Alternating outputs
                    # across rings was tried and REGRESSED: output DMA
                    # instructions interleaved in the Sync engine's
                    # queue block later input-slab issues while waiting
                    # on bias-add semaphores.)
                    eng = nc.sync if i == n_slabs - 2 else nc.scalar
                    eng.dma_start(y[:, o0 : o0 + nt * D_OUT], ys[:])
    nc.compile()
    return nc


def _get_nc(mm_dtype_name):
    if mm_dtype_name not in _cache:
        _cache[mm_dtype_name] = _build(mm_dtype_name)
    return _cache[mm_dtype_name]


def kernel(x, index, weight, bias, _trace=False):
    from concourse.bass_utils import run_bass_kernel_spmd

    x = np.ascontiguousarray(np.asarray(x, dtype=np.float32))
    weight = np.ascontiguousarray(np.asarray(weight, dtype=np.float32))
    bias = np.ascontiguousarray(np.asarray(bias, dtype=np.float32))
    idx = np.asarray(index).astype(np.int64, copy=False)

    ids = [np.nonzero(idx == e)[0] for e in range(N_EXPERTS)]

    in_maps = []
    for e in range(N_EXPERTS):
        n_e = min(len(ids[e]), CAP)
        x_e = np.zeros((CAP, D_IN), dtype=np.float32)
        x_e[:n_e] = x[ids[e][:n_e]]
        # Pack slab-major: xt_e[p, KC*t0 + kc*ts + t] = x_e[t0+t, kc*128+p]
        xt_e = np.empty((128, KC * CAP), dtype=np.float32)
        for t0, ts in SLABS:
            blk = x_e[t0 : t0 + ts].reshape(ts, KC, 128)  # [t, kc, p]
            xt_e[:, KC * t0 : KC * (t0 + ts)] = (
                blk.transpose(2, 1, 0).reshape(128, KC * ts)
            )
        w_e = weight[e]
        head_a_e = np.concatenate(
            [xt_e[:, 0 : KC * HEAD_TOK], w_e[0:128, :], w_e[128:256, :]],
            axis=1,
        )
        head_b_e = np.concatenate([w_e[256:384, :], w_e[384:512, :]], axis=1)
        in_maps.append(
            {
                "xt": xt_e,
                "head_a": np.ascontiguousarray(head_a_e),
                "head_b": np.ascontiguousarray(head_b_e),
                "b": np.ascontiguousarray(
                    np.broadcast_to(bias[e], (128, D_OUT))
                ),
            }
        )

    x_dt_name, _, _ = _DT_MAP[MM_DTYPE]
    cast = {"bfloat16": None, "float16": np.float16, "float32": np.float32,
            "float32r": np.float32}
    ct = cast[x_dt_name]
    if ct is None:
        import ml_dtypes

        ct = ml_dtypes.bfloat16
    in_maps = [
        {k: v.astype(ct) for k, v in m.items()}
        for m in in_maps
    ]

    nc = _get_nc(MM_DTYPE)
    res = run_bass_kernel_spmd(
        nc, in_maps, core_ids=list(range(N_EXPERTS)), trace=_trace
    )

    out = np.empty((x.shape[0], D_OUT), dtype=np.float32)
    for e in range(N_EXPERTS):
        n_e = min(len(ids[e]), CAP)
        # Unpack [p, a_global, o] -> token-major [a_global*128+p, o]
        y_pm = res.results[e]["y"].reshape(128, CAP // 128, D_OUT)
        y_e = y_pm.transpose(1, 0, 2).reshape(CAP, D_OUT)
        out[ids[e][:n_e]] = y_e[:n_e].astype(np.float32)
        if len(ids[e]) > CAP:  # capacity overflow: host fallback (correctness net)
            over = ids[e][CAP:]
            out[over] = x[over] @ weight[e] + bias[e]

    if _trace:
        return out, res

    return out
